# revision 63
# baseline (speedup 1.0000x reference)
"""BERT encoder layer (B=2, S=2048, H=768, NH=12, F=3072) on 8 TRN2 NeuronCores.

Sharding: data-parallel over (batch, query-chunk): core c handles batch c//4,
query rows (c%4)*512 .. +512.  Each core redundantly projects K/V for the full
2048-token sequence of its batch (no collectives).

Structural ideas (vs the 362us baseline):
- Ones-column V: the ctx matmul also produces the softmax denominator Z in
  row 64; ctx rows are staged to SBUF (freeing PSUM early), 1/Z is broadcast
  across 64 partitions via a DRAM round-trip, then scaled on DVE.
- Wa1/Wa2 are column-mean-centered on the host, which makes the pre-LN rows
  exactly zero-mean: LN needs no mean subtraction.  rstd comes from an ACT
  Square+accum pass and a DVE bit-trick rsqrt (1 Newton step) - no Sqrt
  table switch ever; LN application is one ACT Copy(scale=rstd) pass.
- Attention is software-pipelined at depth 1: pair p emits ctx(p-1)
  interleaved with its scores (exp tiles for one full pair stay resident);
  the V projection hides inside pair 0's exp stream; KT(p+1) chunks spread
  across pair p; Wa1's k<=4 chunks cover the last pair's normalization.
- FFN weights are prefetched 8 deep starting at phase 3; the Gelu table load
  is prefetched with a dummy activation right after the last attention exp
  (Square/Copy used by the LNs exist in every ACT table set).

Notes from abandoned experiments (see memory/trn2-bert-encoder-findings.md):
tensor-parallel K/V via AllGather is ~10x slower than documented on this
runner; reciprocal_approx_fast and DVE divide are broken/unsupported here.
"""

import sys

for _p in ("/opt/trn_rl_repo",):
    if _p not in sys.path:
        sys.path.insert(0, _p)

import numpy as np
import ml_dtypes

H = 768
F = 3072
NH = 12
DH = 64
S = 2048
B = 2
QC = 512          # query rows per core
KH = H // 128     # 6 contraction chunks for H
KF = F // 128     # 24 contraction chunks for F
NT = S // 128     # 16 token tiles per sequence
QT = QC // 128    # 4 token tiles per core's query chunk
NPAIR = NH // 2   # 6 head pairs
EPS = 1e-5
RSQRT_MAGIC = 0x5F3759DF

BF = ml_dtypes.bfloat16

_CACHE = {}


def _build_module(act="Gelu", debug_dump=False):
    """act: activation name for the FFN ("Gelu" for real runs; CoreSim lacks
    Gelu so dev-sim uses "Tanh")."""
    import concourse.bass as bass
    import concourse.tile as tile
    from concourse import bacc, mybir
    from concourse.masks import make_identity
    from contextlib import ExitStack

    bf16 = mybir.dt.bfloat16
    f32 = mybir.dt.float32
    i32 = mybir.dt.int32
    ts = bass.ts
    AF = mybir.ActivationFunctionType

    nc = bacc.Bacc("TRN2", target_bir_lowering=False, debug=False)

    d_xT = nc.dram_tensor("xT", [128, KH, S], bf16, kind="ExternalInput").ap()
    d_xqT = nc.dram_tensor("xqT", [128, KH, QC], bf16, kind="ExternalInput").ap()
    d_xqb = nc.dram_tensor("xqb", [128, QT, H], bf16, kind="ExternalInput").ap()
    d_wq = nc.dram_tensor("wq", [128, KH, H], bf16, kind="ExternalInput").ap()
    d_wk = nc.dram_tensor("wk", [128, KH, H], bf16, kind="ExternalInput").ap()
    d_wv = nc.dram_tensor("wv", [128, KH, H], bf16, kind="ExternalInput").ap()
    d_wa1 = nc.dram_tensor("wa1", [128, KH, H], bf16, kind="ExternalInput").ap()
    d_wa2 = nc.dram_tensor("wa2", [128, KH, H], bf16, kind="ExternalInput").ap()
    d_w1 = nc.dram_tensor("w1", [KF, 128, KH, 128], bf16, kind="ExternalInput").ap()
    d_w2 = nc.dram_tensor("w2", [KF, 128, H], bf16, kind="ExternalInput").ap()
    d_out = nc.dram_tensor("out", [128, QT, H], f32, kind="ExternalOutput").ap()
    d_zs = nc.dram_tensor("zscratch", [NPAIR, 2, QC], f32).ap()
    if debug_dump:
        d_dbg_qt = nc.dram_tensor("dbg_qt", [128, KH, QC], bf16,
                                  kind="ExternalOutput").ap()
        d_dbg_kt0 = nc.dram_tensor("dbg_kt0", [128, NT, 128], bf16,
                                   kind="ExternalOutput").ap()
        d_dbg_v = nc.dram_tensor("dbg_v", [128, NT, NH, DH + 1], bf16,
                                 kind="ExternalOutput").ap()
        d_dbg_ctx = nc.dram_tensor("dbg_ctx", [128, KH, QC], bf16,
                                   kind="ExternalOutput").ap()
        d_dbg_an1 = nc.dram_tensor("dbg_an1", [128, QT, H], bf16,
                                   kind="ExternalOutput").ap()

    with tile.TileContext(nc) as tc, ExitStack() as ctx:
        singles = ctx.enter_context(tc.tile_pool(name="singles", bufs=1))

        # ---- resident SBUF tensors; DMA order = need order ----------------
        WQ = singles.tile([128, KH, H], bf16)
        nc.sync.dma_start(out=WQ[:], in_=d_wq[:])
        XQT = singles.tile([128, KH, QC], bf16)
        nc.sync.dma_start(out=XQT[:], in_=d_xqT[:])
        WK = singles.tile([128, KH, H], bf16)
        nc.sync.dma_start(out=WK[:], in_=d_wk[:])
        XT = singles.tile([128, KH, S], bf16)
        nc.sync.dma_start(out=XT[:, :, ts(0, 512)], in_=d_xT[:, :, ts(0, 512)])
        WV = singles.tile([128, KH, H], bf16)
        nc.sync.dma_start(out=WV[:], in_=d_wv[:])
        for n in range(1, 4):
            nc.sync.dma_start(out=XT[:, :, ts(n, 512)], in_=d_xT[:, :, ts(n, 512)])
        WA1 = singles.tile([128, KH, H], bf16)
        nc.sync.dma_start(out=WA1[:], in_=d_wa1[:])
        XQB = singles.tile([128, QT, H], bf16)
        nc.sync.dma_start(out=XQB[:], in_=d_xqb[:])
        WA2 = singles.tile([128, KH, H], bf16)
        nc.sync.dma_start(out=WA2[:], in_=d_wa2[:])

        IDN = singles.tile([128, 128], bf16)
        make_identity(nc, IDN[:])
        MAGIC = singles.tile([128, 1], i32)
        nc.vector.memset(MAGIC[:], RSQRT_MAGIC)


        # V token-major, augmented with a ones column per head: lhsT [V_h | 1]
        # makes the ctx matmul also produce the softmax denominator in row 64.
        V_sb = singles.tile([128, NT, NH, DH + 1], bf16)
        nc.vector.memset(V_sb[:, :, :, DH:DH + 1], 1.0)
        QT_sb = singles.tile([128, KH, QC], bf16)      # Q^T feature-major
        CTX_sb = singles.tile([128, KH, QC], bf16)     # ctx^T feature-major
        AN1B = singles.tile([128, QT, H], bf16)        # an1 token-major
        AN1T = singles.tile([128, KH, QC], bf16)       # an1^T
        FFT_sb = singles.tile([128, KH, QC], bf16)     # ff^T

        def rstd_of(pool, ps_ap, tag):
            """rstd = rsqrt(mean(ps^2) + EPS) for zero-mean rows, [128,1] f32.
            ACT Square+accum for the row sum of squares (Square is in every
            ACT table set - no table switch), then DVE bit-trick rsqrt with
            two Newton steps (no Sqrt table, no iterative divide)."""
            sqs = pool.tile([128, H], bf16, tag=f"{tag}sqs")
            ssq = pool.tile([128, 1], f32, tag=f"{tag}ssq")
            nc.scalar.activation(sqs[:], ps_ap, AF.Square, accum_out=ssq[:])
            ve = pool.tile([128, 1], f32, tag=f"{tag}ve")
            nc.vector.tensor_scalar(
                out=ve[:], in0=ssq[:], scalar1=1.0 / H, scalar2=EPS,
                op0=mybir.AluOpType.mult, op1=mybir.AluOpType.add)
            sh = pool.tile([128, 1], i32, tag=f"{tag}sh")
            nc.vector.tensor_scalar(
                out=sh[:], in0=ve[:].bitcast(i32), scalar1=1, scalar2=None,
                op0=mybir.AluOpType.logical_shift_right)
            seed = pool.tile([128, 1], i32, tag=f"{tag}seed")
            nc.vector.tensor_sub(seed[:], MAGIC[:], sh[:])
            y = seed[:].bitcast(f32)
            for it in range(1):
                t1 = pool.tile([128, 1], f32, tag=f"{tag}t1_{it}")
                nc.vector.tensor_mul(t1[:], y, y)
                nc.vector.tensor_mul(t1[:], t1[:], ve[:])
                nc.vector.tensor_scalar(
                    out=t1[:], in0=t1[:], scalar1=-0.5, scalar2=1.5,
                    op0=mybir.AluOpType.mult, op1=mybir.AluOpType.add)
                yn = pool.tile([128, 1], f32, tag=f"{tag}y_{it}")
                nc.vector.tensor_mul(yn[:], y, t1[:])
                y = yn[:]
            return y

        # ================ Phase 1: Q^T projection =======================
        with tc.tile_pool(name="psA", bufs=2, space="PSUM") as psA:
            for mk in range(KH):
                ps = psA.tile([128, QC], f32, tag="qt")
                for k in range(KH):
                    nc.tensor.matmul(
                        ps[:], lhsT=WQ[:, k, ts(mk, 128)], rhs=XQT[:, k, :],
                        start=(k == 0), stop=(k == KH - 1))
                nc.vector.tensor_copy(QT_sb[:, mk, :], ps[:])

        # ================ Phase 2: attention pipeline ===================
        # pair p: emits ctx(p-1) interleaved with scores(p); V projection
        # rides inside pair 0; KT(p+1) chunks spread across pair p.
        with tc.tile_pool(name="kt", bufs=2) as kt_pool, \
             tc.tile_pool(name="exp", bufs=9) as e_pool, \
             tc.tile_pool(name="rz", bufs=2) as z_pool, \
             tc.tile_pool(name="rbs", bufs=4) as rbs_pool, \
             tc.tile_pool(name="ctmp", bufs=2) as ctmp_pool, \
             tc.tile_pool(name="ps_kt", bufs=2, space="PSUM") as ps_kt, \
             tc.tile_pool(name="ps_sc", bufs=1, space="PSUM") as ps_sc, \
             tc.tile_pool(name="ps_cv", bufs=2, space="PSUM") as ps_cv:

            def emit_kt_chunk(KT, p, n):
                ps = ps_kt.tile([128, 512], f32, tag="pskt", name=f"pskt{p}_{n}")
                for k in range(KH):
                    nc.tensor.matmul(
                        ps[:], lhsT=WK[:, k, ts(p, 128)], rhs=XT[:, k, ts(n, 512)],
                        start=(k == 0), stop=(k == KH - 1))
                nc.vector.tensor_copy(
                    KT[:, 4 * n:4 * (n + 1), :].rearrange("p a b -> p (a b)"),
                    ps[:])

            es = {}
            cur_s = {}

            def emit_scores(p, kc):
                # two kc chunks share one 4-bank PSUM tile so a single exp
                # ACTIVATE covers 2048 elements - halves the ~293ns fixed
                # per-instruction ACT overhead that paces the attention phase
                if kc % 2 == 0:
                    cur_s[p] = ps_sc.tile([128, 4, 512], f32, tag="s",
                                          name=f"s{p}_{kc // 2}")
                s = cur_s[p]
                o = 2 * (kc % 2)
                nc.tensor.matmul(
                    s[:, o, :], lhsT=KTs[p][0:64, kc, :], rhs=QT_sb[0:64, p, :],
                    start=True, stop=True, tile_position=(0, 0))
                nc.tensor.matmul(
                    s[:, o + 1, :], lhsT=KTs[p][64:128, kc, :],
                    rhs=QT_sb[64:128, p, :],
                    start=True, stop=True, tile_position=(64, 0))
                if kc % 2 == 1:
                    e = e_pool.tile([128, 4, 512], bf16, tag="e",
                                    name=f"e{p}_{kc // 2}")
                    nc.scalar.activation(e[:], s[:], AF.Exp)
                    es[(p, kc // 2)] = e

            ctx_acc = {}

            def emit_ctx(p, kc):
                if kc == 0:
                    ctx_acc[p] = (
                        ps_cv.tile([128, QC], f32, tag="cv", name=f"ctxA{p}"),
                        ps_cv.tile([128, QC], f32, tag="cv", name=f"ctxB{p}"))
                ca, cb = ctx_acc[p]
                e = es[(p, kc // 2)]
                o = 2 * (kc % 2)
                nc.tensor.matmul(
                    ca[0:DH + 1, :], lhsT=V_sb[:, kc, 2 * p, :], rhs=e[:, o, :],
                    start=(kc == 0), stop=(kc == NT - 1))
                nc.tensor.matmul(
                    cb[0:DH + 1, :], lhsT=V_sb[:, kc, 2 * p + 1, :],
                    rhs=e[:, o + 1, :],
                    start=(kc == 0), stop=(kc == NT - 1))
                if kc % 2 == 1:
                    es.pop((p, kc // 2))

            def emit_norm(p):
                # softmax normalization: rows 0..63 are sum(exp . v), row 64 is
                # Z = sum(exp).  Stage ctx out of PSUM (frees the banks),
                # reciprocal on DVE, partition-broadcast 1/Z via a DRAM
                # round-trip, scale on DVE.  Head B lands in CTX_sb via an
                # SBUF-SBUF DMA (DVE cannot shift partitions).
                ca, cb = ctx_acc.pop(p)
                cst = ctmp_pool.tile([DH + 1, 2, QC], f32, tag="cst",
                                     name=f"cst{p}")
                nc.vector.tensor_copy(cst[0:DH + 1, 0, :], ca[0:DH + 1, :])
                nc.vector.tensor_copy(cst[0:DH + 1, 1, :], cb[0:DH + 1, :])
                rz = z_pool.tile([DH + 1, 2, QC], f32, tag="rz", name=f"rz{p}")
                nc.vector.reciprocal(rz[DH:DH + 1, :, :], cst[DH:DH + 1, :, :])
                rbsA = rbs_pool.tile([DH, QC], f32, tag="rb", name=f"rbA{p}")
                rbsB = rbs_pool.tile([DH, QC], f32, tag="rb", name=f"rbB{p}")
                nc.sync.dma_start(out=d_zs[p:p + 1], in_=rz[DH:DH + 1, :, :])

                def bcast_dram(src):
                    return bass.AP(
                        tensor=src.tensor, offset=src.offset,
                        ap=[[0, DH]] + list(src.ap))
                nc.sync.dma_start(out=rbsA[:], in_=bcast_dram(d_zs[p, 0, :]))
                nc.sync.dma_start(out=rbsB[:], in_=bcast_dram(d_zs[p, 1, :]))
                nc.vector.tensor_mul(
                    CTX_sb[0:DH, p, :], cst[0:DH, 0, :], rbsA[:])
                ctmp = ctmp_pool.tile([DH, QC], bf16, tag="ctmp", name=f"ctmp{p}")
                nc.vector.tensor_mul(ctmp[:], cst[0:DH, 1, :], rbsB[:])
                nc.sync.dma_start(out=CTX_sb[DH:128, p, :], in_=ctmp[:])

            def emit_v(m):
                # V for token tile m, all heads, via two PSUM halves
                pa = ps_cv.tile([128, 512], f32, tag="cv", name=f"va{m}")
                for k in range(KH):
                    nc.tensor.matmul(
                        pa[:], lhsT=XT[:, k, ts(m, 128)], rhs=WV[:, k, 0:512],
                        start=(k == 0), stop=(k == KH - 1))
                nc.vector.tensor_copy(
                    V_sb[:, m, 0:8, 0:DH],
                    pa[:].rearrange("p (h d) -> p h d", h=8))
                pb = ps_cv.tile([128, 512], f32, tag="cv", name=f"vb{m}")
                for k in range(KH):
                    nc.tensor.matmul(
                        pb[:, 0:256], lhsT=XT[:, k, ts(m, 128)], rhs=WV[:, k, 512:768],
                        start=(k == 0), stop=(k == KH - 1))
                nc.vector.tensor_copy(
                    V_sb[:, m, 8:12, 0:DH],
                    pb[:, 0:256].rearrange("p (h d) -> p h d", h=4))

            KTs = {0: kt_pool.tile([128, NT, 128], bf16, tag="kt", name="kt0")}
            for n in range(4):
                emit_kt_chunk(KTs[0], 0, n)
            if debug_dump:
                nc.sync.dma_start(out=d_dbg_kt0[:], in_=KTs[0][:])

            for p in range(NPAIR):
                if p + 1 < NPAIR:
                    KTs[p + 1] = kt_pool.tile([128, NT, 128], bf16, tag="kt",
                                              name=f"kt{p + 1}")
                for kc in range(NT):
                    if p > 0:
                        emit_ctx(p - 1, kc)
                    emit_scores(p, kc)
                    if p == 0:
                        emit_v(kc)
                    if p + 1 < NPAIR and kc % 4 == 3:
                        emit_kt_chunk(KTs[p + 1], p + 1, kc // 4)
                if p > 0:
                    emit_norm(p - 1)
                    KTs.pop(p - 1, None)

            # prefetch the Gelu ACT table while PE runs Wa1 (Square/Copy used
            # by LN are present in every table set - no further switches)
            gdummy = e_pool.tile([128, 8], bf16, tag="gdummy")
            nc.scalar.activation(gdummy[:], QT_sb[:, 0, 0:8], getattr(AF, act))
            for kc in range(NT):
                emit_ctx(NPAIR - 1, kc)
            emit_norm(NPAIR - 1)
            if debug_dump:
                nc.sync.dma_start(out=d_dbg_qt[:], in_=QT_sb[:])
                nc.sync.dma_start(out=d_dbg_v[:], in_=V_sb[:])
                nc.sync.dma_start(out=d_dbg_ctx[:], in_=CTX_sb[:])

        # ================ Phase 3: Wa1 + LN1 + transpose ================
        with tc.tile_pool(name="w1s", bufs=8) as w1_pool, \
             tc.tile_pool(name="w2s", bufs=8) as w2_pool, \
             tc.tile_pool(name="g", bufs=3) as g_pool, \
             tc.tile_pool(name="ln", bufs=2) as ln_pool:
            w1ts, w2ts = [], []

            def load_w(f):
                w1t = w1_pool.tile([128, KH, 128], bf16, tag="w1t", name=f"w1t{f}")
                nc.sync.dma_start(out=w1t[:], in_=d_w1[f])
                w1ts.append(w1t)
                w2t = w2_pool.tile([128, H], bf16, tag="w2t", name=f"w2t{f}")
                nc.sync.dma_start(out=w2t[:], in_=d_w2[f])
                w2ts.append(w2t)

            with tc.tile_pool(name="proj768b", bufs=4, space="PSUM") as psB:
                for f in range(8):
                    load_w(f)
                pss = []
                for t in range(QT):
                    ps = psB.tile([128, H], f32, tag="ps", name=f"an1pre{t}")
                    pss.append(ps)
                for k in range(KH - 1):
                    for t in range(QT):
                        nc.tensor.matmul(
                            pss[t][:, 0:512], lhsT=CTX_sb[:, k, ts(t, 128)],
                            rhs=WA1[:, k, 0:512], start=(k == 0), stop=False)
                        nc.tensor.matmul(
                            pss[t][:, 512:768], lhsT=CTX_sb[:, k, ts(t, 128)],
                            rhs=WA1[:, k, 512:768], start=(k == 0), stop=False)
                for t in range(QT):
                    k = KH - 1
                    ps = pss[t]
                    nc.tensor.matmul(
                        ps[:, 0:512], lhsT=CTX_sb[:, k, ts(t, 128)],
                        rhs=WA1[:, k, 0:512], start=False, stop=True)
                    nc.tensor.matmul(
                        ps[:, 512:768], lhsT=CTX_sb[:, k, ts(t, 128)],
                        rhs=WA1[:, k, 512:768], start=False, stop=True)
                    rstd = rstd_of(ln_pool, ps[:], f"l1_{t}")
                    lno = ln_pool.tile([128, H], bf16, tag="lnout")
                    nc.scalar.activation(lno[:], ps[:], AF.Copy, scale=rstd)
                    nc.vector.tensor_add(AN1B[:, t, :], lno[:], XQB[:, t, :])
                    tp = psB.tile([128, KH, 128], bf16, tag="ps", name=f"tp{t}")
                    for m in range(KH):
                        nc.tensor.transpose(tp[:, m, :], AN1B[:, t, ts(m, 128)], IDN[:])
                    nc.vector.tensor_copy(AN1T[:, :, ts(t, 128)], tp[:])
                if debug_dump:
                    nc.sync.dma_start(out=d_dbg_an1[:], in_=AN1B[:])

            # ================ Phase 4: FFN ==============================
            with tc.tile_pool(name="ps_ff", bufs=1, space="PSUM") as ps_ff, \
                 tc.tile_pool(name="ps_h1", bufs=2, space="PSUM") as ps_h1:
                ffps = []
                for m in range(KH):
                    ffps.append(ps_ff.tile([128, QC], f32, tag=f"ff{m}",
                                           name=f"ff{m}"))

                def emit_h1(f):
                    if f + 8 < KF:
                        load_w(f + 8)
                    h1 = ps_h1.tile([128, QC], f32, tag="h1", name=f"h1_{f}")
                    for k in range(KH):
                        nc.tensor.matmul(
                            h1[:], lhsT=w1ts[f][:, k, :], rhs=AN1T[:, k, :],
                            start=(k == 0), stop=(k == KH - 1))
                    return h1

                pend_h1 = emit_h1(0)
                for f in range(KF):
                    h1 = pend_h1
                    g = g_pool.tile([128, QC], bf16, tag="g", name=f"g{f}")
                    nc.scalar.activation(g[:], h1[:], getattr(AF, act))
                    if f + 1 < KF:
                        pend_h1 = emit_h1(f + 1)
                    for m in range(KH):
                        nc.tensor.matmul(
                            ffps[m][:], lhsT=w2ts[f][:, ts(m, 128)], rhs=g[:],
                            start=(f == 0), stop=(f == KF - 1))
                        if f == KF - 1:
                            nc.vector.tensor_copy(FFT_sb[:, m, :], ffps[m][:])

            # ================ Phase 5: Wa2 + LN2 + out ==================
            with tc.tile_pool(name="proj768c", bufs=2, space="PSUM") as psC, \
                 tc.tile_pool(name="outp", bufs=2) as out_pool:
                for t in range(QT):
                    ps = psC.tile([128, H], f32)
                    for k in range(KH):
                        nc.tensor.matmul(
                            ps[:, 0:512], lhsT=FFT_sb[:, k, ts(t, 128)],
                            rhs=WA2[:, k, 0:512], start=(k == 0), stop=(k == KH - 1))
                    for k in range(KH):
                        nc.tensor.matmul(
                            ps[:, 512:768], lhsT=FFT_sb[:, k, ts(t, 128)],
                            rhs=WA2[:, k, 512:768], start=(k == 0), stop=(k == KH - 1))
                    rstd = rstd_of(ln_pool, ps[:], f"l2_{t}")
                    lno = ln_pool.tile([128, H], f32, tag="lnof")
                    nc.scalar.activation(lno[:], ps[:], AF.Copy, scale=rstd)
                    ot = out_pool.tile([128, H], f32)
                    nc.vector.tensor_add(ot[:], lno[:], AN1B[:, t, :])
                    nc.sync.dma_start(out=d_out[:, t, :], in_=ot[:])

    nc.compile()
    return nc


def _numpy_fallback(x, Wq, bq, Wk, bk, Wv, bv, Wa1, ba1, g1, be1,
                    W1, b1, W2, b2, Wa2, ba2, g2, be2):
    from scipy.special import erf

    def ln(v, g, b):
        mu = v.mean(-1, keepdims=True)
        var = ((v - mu) ** 2).mean(-1, keepdims=True)
        return (v - mu) / np.sqrt(var + EPS) * g + b

    out = np.zeros_like(x)
    for bi in range(x.shape[0]):
        xb = x[bi]
        q = (xb @ Wq + bq).reshape(S, NH, DH)
        k = (xb @ Wk + bk).reshape(S, NH, DH)
        v = (xb @ Wv + bv).reshape(S, NH, DH)
        ctx = np.zeros((S, NH, DH), np.float32)
        for h in range(NH):
            s = (q[:, h, :] @ k[:, h, :].T) / np.sqrt(np.float32(DH))
            s = s - s.max(-1, keepdims=True)
            e = np.exp(s)
            ctx[:, h, :] = (e / e.sum(-1, keepdims=True)) @ v[:, h, :]
        an1 = ln(ctx.reshape(S, H) @ Wa1 + ba1, g1, be1) + xb
        hh = an1 @ W1 + b1
        gg = hh * 0.5 * (1.0 + erf(hh / np.sqrt(2.0)))
        ff = gg @ W2 + b2
        out[bi] = ln(ff @ Wa2 + ba2, g2, be2) + an1
    return out


def _prep_inputs(inputs):
    """Host-side shard + cast.  Returns list of 8 in_maps."""
    x = np.asarray(inputs["x"], np.float32)

    def wtile(w):
        # [768, O] -> [128, 6, O] partition-major bf16
        return np.ascontiguousarray(
            w.reshape(KH, 128, -1).transpose(1, 0, 2)).astype(BF)

    def center(w):
        # subtract per-row output-mean: makes pre-LN rows exactly zero-mean
        return w - w.mean(axis=1, keepdims=True)

    wq = wtile(np.asarray(inputs["Wq"], np.float32) / np.sqrt(np.float32(DH)))
    wk = wtile(np.asarray(inputs["Wk"], np.float32))
    wv = wtile(np.asarray(inputs["Wv"], np.float32))
    wa1 = wtile(center(np.asarray(inputs["Wa1"], np.float32)))
    wa2 = wtile(center(np.asarray(inputs["Wa2"], np.float32)))
    w1 = np.ascontiguousarray(
        np.asarray(inputs["W1"], np.float32)
        .reshape(KH, 128, KF, 128).transpose(2, 1, 0, 3)).astype(BF)
    w2 = np.ascontiguousarray(
        np.asarray(inputs["W2"], np.float32).reshape(KF, 128, H)).astype(BF)

    in_maps = []
    for c in range(8):
        b, qi = divmod(c, 4)
        xb = x[b]                                        # [S, H]
        xT = np.ascontiguousarray(
            xb.T.reshape(KH, 128, S).transpose(1, 0, 2)).astype(BF)
        xqT = np.ascontiguousarray(xT[:, :, qi * QC:(qi + 1) * QC])
        xqb = np.ascontiguousarray(
            xb[qi * QC:(qi + 1) * QC].reshape(QT, 128, H).transpose(1, 0, 2)
        ).astype(BF)
        in_maps.append(dict(
            xT=xT, xqT=xqT, xqb=xqb, wq=wq, wk=wk, wv=wv, wa1=wa1, wa2=wa2,
            w1=w1, w2=w2))
    return in_maps


def kernel(**inputs):
    # Generic fallback: the device fast-path assumes zero biases and unit
    # layernorm gains (true for this model's weights).
    zero_keys = ["bq", "bk", "bv", "ba1", "be1", "b1", "b2", "ba2", "be2"]
    if any(np.any(np.asarray(inputs[k]) != 0) for k in zero_keys) or \
       np.any(np.asarray(inputs["g1"]) != 1) or np.any(np.asarray(inputs["g2"]) != 1):
        return _numpy_fallback(
            **{k: np.asarray(v, np.float32) for k, v in inputs.items()})

    from concourse.bass_utils import run_bass_kernel_spmd

    if "nc" not in _CACHE:
        _CACHE["nc"] = _build_module()
    nc = _CACHE["nc"]

    in_maps = _prep_inputs(inputs)
    res = run_bass_kernel_spmd(nc, in_maps, core_ids=list(range(8)))
    out = np.zeros((B, S, H), np.float32)
    for c in range(8):
        b, qi = divmod(c, 4)
        o = res.results[c]["out"]                        # [128, QT, H]
        out[b, qi * QC:(qi + 1) * QC] = o.transpose(1, 0, 2).reshape(QC, H)
    return out
